# revision 45
# baseline (speedup 1.0000x reference)
"""Trainium2 Bass kernel for the two-branch softmax MLP + diffminmaxprob join.

Reference computation (per batch row r):
    a = softmax(relu(x @ W1a) @ W2a)   # [512]  (all reference biases are 0)
    b = softmax(relu(x @ W1b) @ W2b)   # [512]
    out[v] = max_{i-j+511=v} min(a_i, b_j)         # v in [0, 1022]

Sharding: the 1023 output diagonals are strided across the 8 cores
(core c owns diagonals t with t % 8 == c).  Every core runs an IDENTICAL
instruction stream (true SPMD); the per-core diagonal offset is encoded
purely in the data by permuting W2b's columns per core and appending 8
dummy columns whose bias is -60000 (=> exactly-zero softmax probs).

Precision: everything flows in fp16 (weights, x, h, probs) with fp32 PSUM
accumulation and fp32 exp/sum.  fp16 matmuls run at 1 cycle/row on the PE
(4x over fp32) and fp16 min/max tensor_tensor ops hit the DVE 2x_1p mode
(2x over fp32).  Measured end-to-end rel err vs the fp32 reference is
~8e-4, far inside the 2e-2 gate.  Logits are bounded (|logit| < 1.5), so
the softmax skips the max-subtraction pass entirely.

The join processes merged family pairs: fam1 unit j0 (window l = 512-8*j0)
and its complementary fam2 unit 56-j0 (window l-1) share one [P, 2, 8, l]
instruction chain - a single fp16 tensor_tensor(min) over sliding-window
access patterns of the zero-padded b-probs, then an in-place fp16
tensor_tensor(max) fold chain (each fold halves the window; odd lengths
overlap one element - harmless for max) down to <=16 columns, finished by
one tensor_reduce(max) that scatters both families' 8-column blocks into
the output tile.  Weight/input DMAs issue from three engine queues
(SP / Pool / Act) so the transfers run concurrently; W2a/W2b load as
column-halves so each half-softmax's exp can start as early as possible.
"""

import numpy as np

import concourse.bass as bass
import concourse.bacc as bacc
import concourse.mybir as mybir
from concourse import tile
from concourse.bass_types import AP as BassAP
from concourse.bass_utils import run_bass_kernel_spmd

F32 = mybir.dt.float32
F16 = mybir.dt.float16
AF = mybir.ActivationFunctionType
ALU = mybir.AluOpType
AX = mybir.AxisListType

B = 256          # batch
D = 1024         # hidden / input dim
S = 512          # softmax size
SP = S + 8       # padded branch-b softmax size (8 dummy -inf columns)
P = 128          # partitions
NCORES = 8
KT = D // P      # 8 contraction tiles
RB = B // P      # 2 row blocks
J = S // NCORES  # 64 diagonal slots per family per core

GJ = 8                        # diagonals per grouped join instruction
LEAD = 8 * (GJ - 1)           # 56: left zero pad before the b-prob window
BW = LEAD + SP + 8 * GJ       # 640: padded b-prob width


def build_nc():
    nc = bacc.Bacc(None)

    xt_d = nc.dram_tensor("xt", [D, B], F16, kind="ExternalInput")
    w1a_d = nc.dram_tensor("W1a", [D, D], F16, kind="ExternalInput")
    w1b_d = nc.dram_tensor("W1b", [D, D], F16, kind="ExternalInput")
    w2a_d = nc.dram_tensor("W2a", [D, S], F16, kind="ExternalInput")
    w2b_d = nc.dram_tensor("W2b", [D, SP], F16, kind="ExternalInput")
    b2b_d = nc.dram_tensor("b2b", [1, SP], F16, kind="ExternalInput")
    out_d = nc.dram_tensor("out", [B, 2 * J], F16, kind="ExternalOutput")

    with tile.TileContext(nc) as tc:
        with (
            tc.tile_pool(name="consts", bufs=1) as consts,
            tc.tile_pool(name="wpool", bufs=1) as wpool,
            tc.tile_pool(name="hpool", bufs=1) as hpool,
            tc.tile_pool(name="probs", bufs=1) as probs,
            tc.tile_pool(name="small", bufs=4) as small,
            tc.tile_pool(name="scratch", bufs=6) as scratch,
            tc.tile_pool(name="outp", bufs=1) as outp,
            tc.tile_pool(name="psh", bufs=1, space="PSUM") as psh,
            tc.tile_pool(name="psl", bufs=2, space="PSUM") as psl_pool,
        ):
            ones1 = consts.tile([1, P], F16)
            nc.gpsimd.memset(ones1[:], 1.0)

            # b-prob tiles with zeroed lead/tail padding (memset early)
            bpz = []
            for rb in range(RB):
                t = probs.tile([P, BW], F16, tag=f"bprob{rb}", name=f"bprob{rb}")
                nc.gpsimd.memset(t[:, :LEAD], 0.0)
                nc.gpsimd.memset(t[:, LEAD + SP:], 0.0)
                bpz.append(t)
            # a-probs padded with 8 zero columns (read by the merged fam2
            # windows at their one-cell length padding)
            at = []
            for rb in range(RB):
                t = probs.tile([P, S + 8], F16, tag=f"aprob{rb}",
                               name=f"aprob{rb}")
                nc.gpsimd.memset(t[:, S:], 0.0)
                at.append(t)

            # ---- DMAs, issued from three engine queues so the transfers
            # run concurrently: SP takes xt + W1a, Pool takes W1b, Act takes
            # W2a/W2b (Act is idle until the first relu ~9us in).
            xt = consts.tile([P, KT, B], F16, tag="xt")
            kh = KT // 2

            def dma_xt_half(eng, h):
                eng.dma_start(
                    xt[:, h * kh:(h + 1) * kh, :],
                    xt_d[h * kh * P:(h + 1) * kh * P, :].rearrange(
                        "(k p) r -> p k r", p=P))

            def load_k_tiles(eng, dram, width, name, k_lo, k_hi, kchunk=1):
                ts = []
                for k0 in range(k_lo, k_hi, kchunk):
                    t = wpool.tile([P, kchunk, width], F16,
                                   tag=f"{name}{k0}", name=f"{name}{k0}")
                    eng.dma_start(
                        t[:], dram[k0 * P:(k0 + kchunk) * P, :].rearrange(
                            "(k p) d -> p k d", p=P))
                    for kk in range(kchunk):
                        ts.append(t[:, kk, :])
                return ts

            # Three concurrent DMA queues (SP / Pool / Act), each W1 matrix
            # split across two of them, tiles ordered by when the PE needs
            # them (b-branch k-tiles first, W2a last).
            w1b = load_k_tiles(nc.gpsimd, w1b_d, D, "w1b", 0, 6)
            w1b += load_k_tiles(nc.scalar, w1b_d, D, "w1b", 6, KT)
            dma_xt_half(nc.sync, 0)
            dma_xt_half(nc.sync, 1)
            w1a = load_k_tiles(nc.sync, w1a_d, D, "w1a", 0, 6)
            w1a += load_k_tiles(nc.gpsimd, w1a_d, D, "w1a", 6, KT)
            # W2b as two column-halves + the 8 padded columns, so the
            # b-softmax exp can start on the first half early.
            w2b_h = []
            for h in range(2):
                t = wpool.tile([P, KT, S // 2], F16, tag=f"w2bh{h}",
                               name=f"w2bh{h}")
                nc.scalar.dma_start(
                    t[:], w2b_d[:, h * (S // 2):(h + 1) * (S // 2)].rearrange(
                        "(k p) d -> p k d", p=P))
                w2b_h.append(t)
            w2b8 = wpool.tile([P, KT, SP - S], F16, tag="w2b8", name="w2b8")
            nc.scalar.dma_start(
                w2b8[:], w2b_d[:, S:].rearrange("(k p) d -> p k d", p=P))
            b2row = consts.tile([1, SP], F16, tag="b2row")
            nc.gpsimd.dma_start(b2row[:], b2b_d[:, :])
            # W2a as two column-halves (SP queue tail): each psl_a half-
            # softmax needs ALL k-tiles of its columns, so column-major
            # loading unblocks the first half ~1.6us earlier.
            w2a_h = []
            for h, eng in ((0, nc.gpsimd), (1, nc.sync)):
                t = wpool.tile([P, KT, S // 2], F16, tag=f"w2ah{h}",
                               name=f"w2ah{h}")
                eng.dma_start(
                    t[:], w2a_d[:, h * (S // 2):(h + 1) * (S // 2)].rearrange(
                        "(k p) d -> p k d", p=P))
                w2a_h.append(t)

            # ---- h = relu(x @ W1), one rowblock and branch at a time ------
            # m-groups accumulate k-interleaved (consuming each W1 k-tile the
            # moment its DMA lands).  PSUM zero regions are whole 2KB banks;
            # with 128-wide (one-rowblock) outputs four m-slices pack into
            # one bank, so only the first matmul touching a bank zeroes it
            # (start=True) and only the last closes it (stop=True).
            ht_a = hpool.tile([P, KT, B], F16, tag="ht_a", name="ht_a")
            ht_b = hpool.tile([P, KT, B], F16, tag="ht_b", name="ht_b")

            def make_ht_rb(w1, ht, name, rb):
                pst = psh.tile([P, KT, P], F32, tag=f"pst_{name}",
                               name=f"pst_{name}{rb}")
                for k in range(KT):
                    for m in range(KT):
                        nc.tensor.matmul(
                            pst[:, m, :], w1[k][:, m * P:(m + 1) * P],
                            xt[:, k, rb * P:(rb + 1) * P],
                            start=(k == 0 and m % 4 == 0),
                            stop=(k == KT - 1 and m % 4 == 3))
                if rb == 0:
                    # rowblock 0's relu on the (idle) DVE keeps the serial
                    # Act engine free for the exp chain that gates the join
                    nc.vector.tensor_scalar(ht[:, :, rb * P:(rb + 1) * P],
                                            pst[:], 0.0, None, op0=ALU.max)
                else:
                    # rowblock 1's relu on Act: it must not enter the DVE
                    # stream ahead of the rowblock-0 join units
                    nc.scalar.activation(ht[:, :, rb * P:(rb + 1) * P],
                                         pst[:], AF.Relu)

            # ---- per-rowblock softmax (no max-subtraction; logits ~ +-1.5)
            def softmax_a(rb):
                # two half-width PSUM groups: exp of the first half overlaps
                # the PE finishing the second half
                ssums = []
                for h in range(2):
                    hs = slice(h * (S // 2), (h + 1) * (S // 2))
                    ps = psl_pool.tile([P, S], F32, tag="ps",
                                       name=f"psa{rb}{h}")
                    for m in range(KT):
                        nc.tensor.matmul(ps[:, :S // 2],
                                         ht_a[:, m, rb * P:(rb + 1) * P],
                                         w2a_h[h][:, m, :], start=(m == 0),
                                         stop=(m == KT - 1))
                    ssum = small.tile([P, 1], F32, tag=f"ssum{h}",
                                      name=f"ssum{h}")
                    nc.scalar.activation(at[rb][:, hs], ps[:, :S // 2], AF.Exp,
                                         accum_out=ssum[:])
                    ssums.append(ssum)
                nc.vector.tensor_add(ssums[0][:], ssums[0][:], ssums[1][:])
                rec = small.tile([P, 1], F32, tag="rec")
                nc.vector.reciprocal(rec[:], ssums[0][:])
                nc.vector.tensor_scalar(at[rb][:, :S], at[rb][:, :S], rec[:],
                                        None, op0=ALU.mult)

            def softmax_b(rb):
                bp = bpz[rb][:, LEAD:LEAD + SP]
                ssums = []
                # two column-halves + the 8 padded columns; bias matmuls add
                # the -60000 dummy-column markers (data-encoded, per core)
                for h in range(2):
                    hs = slice(h * (S // 2), (h + 1) * (S // 2))
                    ps = psl_pool.tile([P, S], F32, tag="ps",
                                       name=f"psb{rb}{h}")
                    for m in range(KT):
                        nc.tensor.matmul(ps[:, :S // 2],
                                         ht_b[:, m, rb * P:(rb + 1) * P],
                                         w2b_h[h][:, m, :], start=(m == 0),
                                         stop=False)
                    nc.tensor.matmul(ps[:, :S // 2], ones1[:], b2row[:, hs],
                                     start=False, stop=True)
                    ssum = small.tile([P, 1], F32, tag=f"ssumb{h}",
                                      name=f"ssumb{h}")
                    nc.scalar.activation(bp[:, hs], ps[:, :S // 2], AF.Exp,
                                         accum_out=ssum[:])
                    ssums.append(ssum)
                ps8 = psl_pool.tile([P, SP - S], F32, tag="ps8",
                                    name=f"psb8{rb}")
                for m in range(KT):
                    nc.tensor.matmul(ps8[:], ht_b[:, m, rb * P:(rb + 1) * P],
                                     w2b8[:, m, :], start=(m == 0), stop=False)
                nc.tensor.matmul(ps8[:], ones1[:], b2row[:, S:],
                                 start=False, stop=True)
                ssum8 = small.tile([P, 1], F32, tag="ssumb8")
                nc.scalar.activation(bp[:, S:], ps8[:], AF.Exp,
                                     accum_out=ssum8[:])
                nc.vector.tensor_add(ssums[0][:], ssums[0][:], ssums[1][:])
                nc.vector.tensor_add(ssums[0][:], ssums[0][:], ssum8[:])
                rec = small.tile([P, 1], F32, tag="recb")
                nc.vector.reciprocal(rec[:], ssums[0][:])
                nc.vector.tensor_scalar(bp[:], bp[:], rec[:], None,
                                        op0=ALU.mult)

            # PE warm-up: a few dummy matmuls so the p-state ramp clock
            # starts before the first weight tile lands.
            warm = psh.tile([P, P], F32, tag="pst_b", name="warm")
            for _ in range(14):
                nc.tensor.matmul(warm[:], ones1[:], ones1[:],
                                 start=True, stop=True)

            for rb in range(RB):
                make_ht_rb(w1b, ht_b, "b", rb)
                make_ht_rb(w1a, ht_a, "a", rb)
                softmax_b(rb)   # branch b first: its exp chain is longer, so
                softmax_a(rb)   # it must not queue behind exp_a on the Act engine

            # ---- the join -------------------------------------------------
            # Core c (in the W2b permutation) owns:
            #   family 1 slot j:  v = 511 - 8j - c   (diag t = 8j + c)
            #   family 2 slot j:  v = 1023 - 8j - c
            # bpz content: bpz[LEAD+p] = b[p + c - 7] for p in [7-c, 519-c).
            #
            # Family pairing: fam1 unit j0 (window l = 512-8*j0) merges with
            # fam2 unit js = 56-j0 (window 511-8*j0 = l-1, padded by one
            # harmless cell) into ONE [P, 2, GJ, l] instruction chain:
            #   fam1 slot g: sc[p,0,g,i] = min(at[i],        bpz[LEAD+8*j0+7+8g+i])
            #   fam2 slot g: sc[p,1,g,i] = min(at[1+8*j0+i], bpz[8g+i])
            # (at is padded with zeros at [S:S+8] so fam2's i = l-1 reads 0.)
            o_all = [outp.tile([P, 2 * J], F16, tag=f"o_{rb}", name=f"o_{rb}")
                     for rb in range(RB)]
            for rb in range(RB):
                att, bpt = at[rb], bpz[rb]
                for j0 in range(0, J, GJ):
                    l = S - 8 * j0
                    js = 56 - j0          # fam2 unit start slot
                    f1b = LEAD + 8 * j0 + 7   # fam1 window base in bpz
                    sc = scratch.tile([P, 2 * GJ * S], F16, tag="sc",
                                      name="sc")
                    sc4 = sc[:, :2 * GJ * l].rearrange(
                        "p (f g l) -> p f g l", f=2, g=GJ)
                    in0 = BassAP(tensor=att[:].tensor, offset=att[:].offset,
                                 ap=[tuple(att[:].ap[0]), (1 + 8 * j0, 2),
                                     (0, GJ), (1, l)])
                    base1 = bpt[:, f1b:]
                    in1 = BassAP(tensor=base1.tensor, offset=base1.offset,
                                 ap=[tuple(base1.ap[0]), (-f1b, 2),
                                     (8, GJ), (1, l)])
                    nc.vector.tensor_tensor(out=sc4, in0=in0, in1=in1,
                                            op=ALU.min)
                    cur = l
                    while cur > 16:
                        nxt = (cur + 1) // 2
                        nc.vector.tensor_tensor(
                            out=sc4[:, :, :, :nxt], in0=sc4[:, :, :, :nxt],
                            in1=sc4[:, :, :, cur - nxt:cur], op=ALU.max)
                        cur = nxt
                    ob = o_all[rb][:, j0:]
                    oap = BassAP(tensor=ob.tensor, offset=ob.offset,
                                 ap=[tuple(ob.ap[0]), (J + js - j0, 2),
                                     (1, GJ)])
                    nc.vector.tensor_reduce(oap, sc4[:, :, :, :cur],
                                            axis=AX.X, op=ALU.max)
                nc.sync.dma_start(out_d[rb * P:(rb + 1) * P, :], o_all[rb][:])

    nc.compile()
    return nc


def _prep_core_inputs(inputs, c):
    """Per-core fp16 inputs: transposed x, permuted/padded W2b + bias row."""
    w2b = np.asarray(inputs["W2b"], np.float32)
    w2bp = np.zeros((D, SP), np.float16)
    b2bp = np.full((SP,), -60000.0, np.float16)
    p = np.arange(7 - c, 519 - c)          # padded positions of real cols
    src = p + c - 7                        # = 0..511
    w2bp[:, p] = w2b[:, src].astype(np.float16)
    b2bp[p] = 0.0
    return {
        "xt": np.ascontiguousarray(
            np.asarray(inputs["x"], np.float32).T.astype(np.float16)),
        "W1a": np.asarray(inputs["W1a"], np.float32).astype(np.float16),
        "W1b": np.asarray(inputs["W1b"], np.float32).astype(np.float16),
        "W2a": np.asarray(inputs["W2a"], np.float32).astype(np.float16),
        "W2b": w2bp,
        "b2b": np.ascontiguousarray(b2bp[None, :]),
    }


def assemble(results):
    """Map per-core [B, 128] outputs back to the full [B, 1023] tensor."""
    full = np.empty((B, 2 * S - 1), np.float32)
    js = np.arange(J)
    for c in range(NCORES):
        r = np.asarray(results[c]["out"]).astype(np.float32)
        full[:, 511 - 8 * js - c] = r[:, :J]
        hi_js = js if c > 0 else js[1:]
        full[:, 1023 - 8 * hi_js - c] = r[:, J + hi_js]
    return full


_NC_CACHE = {}


def kernel(**inputs):
    if "nc" not in _NC_CACHE:
        _NC_CACHE["nc"] = build_nc()
    nc = _NC_CACHE["nc"]
    in_maps = [_prep_core_inputs(inputs, c) for c in range(NCORES)]
    res = run_bass_kernel_spmd(nc, in_maps, core_ids=list(range(NCORES)))
    return assemble(res.results)
